# revision 63
# baseline (speedup 1.0000x reference)
"""Multi-head cross-attention kernel for 8 Trainium2 NeuronCores.

Sharding: core = (batch, head-group) - cores 0-3 take batch 0, cores 4-7
batch 1; core m%4 takes heads [4*(m%4), 4*(m%4)+4). Each core projects
q/k/v for its 4 heads, runs fused (no-max) softmax attention fully
on-chip, and produces a partial out-projection (transposed). The host
sums the four per-batch partials and transposes back.

Shapes (hardcoded per problem spec):
  query_states [2, 2048, 1024], key/value_states [2, 4096, 1024],
  Wq/Wk/Wv/Wo [1024, 1024] (torch Linear layout, applied as x @ W.T).

Design notes (per core; b = batch, s = 256-dim hidden slice):
  All matmul operands are bf16 (halves DMA + SBUF; PE rate unchanged).
  PSUM accumulation stays fp32. Activations arrive host-packed so every
  input DMA reads 8 KB (4 KB for v) contiguous per partition - the HW
  DGE queues are descriptor-rate limited, so line size sets bandwidth.
  QT per-head [128, 4, 2048] with pair-partner rows zeroed (full-K=128
  score matmuls keep the PE HAM-warm); KT pair-packed [128, 2, 4096];
  Vsb [kv, 4 heads x 65] with a trailing ones column per head so the
  attn@V accumulation also produces the softmax denominator in PSUM
  row 64. scores.T tiles [128 kv, 1024 q] -> exp on the scalar engine
  (scale folded in; no max subtraction: |scores/8| < 4 here) ->
  X += V'.T @ expS in PSUM.
  Normalization is OUT of the attention loop: per (head, q-half) group
  the denominator row is staged to SBUF and DMA'd into a per-q-half
  [4, 1024] collector; after a q-half finishes, one batched
  reciprocal_approx_accurate + K=4 selection matmuls (0/1 matrix
  broadcasts each head's reciprocal row across its 64 partitions; PE
  operands must start at partition 0/32/64 so single-row reads at
  partition g are illegal) + one [128, 1024] multiply per hc normalize
  Xraw into Xn, then outT = woT.T @ Xn for that q-half. The q-half-0
  tail is emitted a few chunks into q-half-1's attention so the scalar
  engine keeps streaming exps while the PE takes the tail detour.
  outT is bf16; the host sums the four partials per batch in fp32.
"""

import ml_dtypes
import numpy as np

import concourse.tile as tile
from concourse import bacc, mybir
from concourse.bass_utils import run_bass_kernel_spmd

B, QL, KVL, HIDDEN = 2, 2048, 4096, 1024
N_HEADS, HEAD_DIM = 16, 64
SCALE = HEAD_DIM**-0.5
N_CORES = 8
HPC = 4  # heads per core
DS = HPC * HEAD_DIM  # 256: per-core hidden slice

F32 = mybir.dt.float32
F32R = mybir.dt.float32r
BF16 = mybir.dt.bfloat16
I32 = mybir.dt.int32
NP_BF16 = ml_dtypes.bfloat16

HC = HIDDEN // 128  # 8 contraction chunks over hidden
DC = DS // 128  # 2 chunks over the per-core 256-dim slice
KVC = KVL // 128  # 32 kv chunks

# Corrected-Schraudolph exp for the DVE/Pool engines (the scalar engine is
# the softmax bottleneck; every 4th kv chunk's exp runs elsewhere).
#   y  = int32(x*EXP_A' + EXP_B)            (int32 write rounds)
#   z  = bitcast((y & 0x007FFFFF) | 0x3F800000)   = 1 + frac
#   out = bitcast(y) * ((z*EXP_C1 + EXP_C2)*z + 1)
# Minimax-fit constants: max rel err 0.53% (error is periodic in x/ln2).
EXP_A = 2**23 / np.log(2)  # * SCALE folded in at the call site
EXP_B = 1.06973393e9
EXP_C1 = 0.157365897
EXP_C2 = -0.464744215
MASK_M = 0x007FFFFF  # mantissa mask
MASK_E = 0x3F800000  # exponent bits of 1.0f


def _build_program():
    nc = bacc.Bacc(None)
    # host-packed activations: (p, [outer], quad j, i, line)
    xq = nc.dram_tensor("xq", [128, 4, 2, 2048], BF16, kind="ExternalInput")
    xk = nc.dram_tensor("xk", [128, 2, 4, 2, 2048], BF16, kind="ExternalInput")
    xv = nc.dram_tensor("xv", [128, 4, 4, 2, 1024], BF16, kind="ExternalInput")
    wqT = nc.dram_tensor("wqT", [128, HC, DS], BF16, kind="ExternalInput")
    wkT = nc.dram_tensor("wkT", [128, HC, DS], BF16, kind="ExternalInput")
    wvT = nc.dram_tensor("wvT", [128, HC, DS], BF16, kind="ExternalInput")
    woT = nc.dram_tensor("woT", [128, DC, HIDDEN], BF16, kind="ExternalInput")
    selD = nc.dram_tensor("selD", [4, 2, 128], F32R, kind="ExternalInput")
    outT = nc.dram_tensor("outT", [HIDDEN, QL], BF16, kind="ExternalOutput")

    with tile.TileContext(nc) as tc:
        with (
            tc.tile_pool(name="persist", bufs=1) as persist,
            tc.tile_pool(name="wpool", bufs=1) as wpool,
        ):
            # Long-lived SBUF tensors.
            KT = persist.tile([128, DC, KVL], BF16)
            QT = persist.tile([128, HPC, QL], BF16)
            Vsb = persist.tile([128, KVC, HPC, HEAD_DIM + 1], BF16)
            Xraw = persist.tile([128, DC, QL], BF16)  # unnormalized attn out (.T)
            Xn = persist.tile([128, DC, QL], BF16)  # normalized
            wo_sb = persist.tile([128, DC, HIDDEN], BF16)
            # per-q-half denominator collectors (row = head)
            denC = [persist.tile([4, 1024], F32, name=f"denC{i}") for i in range(2)]
            recA = [persist.tile([4, 1024], F32, name=f"recA{i}") for i in range(2)]
            # f32r copies for the broadcast matmul (the PE wants f32r-rounded
            # producers; a convert-copy satisfies the verifier)
            recR = [persist.tile([4, 1024], F32R, name=f"recR{i}") for i in range(2)]
            recS = persist.tile([4, 1024], F32)  # recip scratch
            selC = persist.tile([4, 2, 128], F32R)
            # int constants for the bitwise stage of the fast exp
            mskM = persist.tile([128, 1], I32, name="mskM")
            mskE = persist.tile([128, 1], I32, name="mskE")
            nc.vector.memset(QT, 0.0)
            nc.vector.memset(mskM, MASK_M)
            nc.vector.memset(mskE, MASK_E)

            wq_sb = wpool.tile([128, HC, DS], BF16, tag="wq")
            wk_sb = wpool.tile([128, HC, DS], BF16, tag="wk")
            wv_sb = wpool.tile([128, HC, DS], BF16, tag="wv")
            # wk chunk 0 first, alone: the very first K-proj matmul needs
            # only it, so it shouldn't wait for the full 0.5 MB.
            nc.sync.dma_start(wk_sb[:, 0, :], wkT[:, 0, :])
            nc.sync.dma_start(wk_sb[:, 1:, :], wkT[:, 1:, :])
            nc.scalar.dma_start(wv_sb[:], wvT[:])
            nc.scalar.dma_start(wq_sb[:], wqT[:])
            nc.scalar.dma_start(wo_sb[:], woT[:])
            nc.sync.dma_start(selC[:], selD[:])

            # ones column of V' (softmax denominator accumulator)
            nc.vector.memset(Vsb[:, :, :, HEAD_DIM : HEAD_DIM + 1], 1.0)

            # ---- K projection: KT[dk, kv] = sum_h wkT[h, dk] * xk[h, kv] ----
            with (
                tc.tile_pool(name="xstream", bufs=3) as xs,
                tc.tile_pool(name="pproj", bufs=8, space="PSUM") as pp,
            ):
                for half in range(2):
                    ps = [
                        [pp.tile([128, 512], F32, tag="psk", name=f"psk_{dk}_{t}") for t in range(4)]
                        for dk in range(DC)
                    ]
                    for j in range(4):  # h-chunk pairs
                        xt = xs.tile([128, 2, 2048], BF16, tag="xk")
                        eng = nc.sync if j % 2 == 0 else nc.scalar
                        eng.dma_start(xt[:], xk[:, half, j, :, :])
                        for i in range(2):
                            h = j * 2 + i
                            for dk in range(DC):
                                for t in range(4):
                                    nc.tensor.matmul(
                                        ps[dk][t][:],
                                        wk_sb[:, h, dk * 128 : (dk + 1) * 128],
                                        xt[:, i, t * 512 : (t + 1) * 512],
                                        start=(h == 0),
                                        stop=(h == HC - 1),
                                    )
                    for dk in range(DC):
                        for t in range(4):
                            nc.vector.tensor_copy(
                                KT[:, dk, half * 2048 + t * 512 : half * 2048 + (t + 1) * 512],
                                ps[dk][t][:],
                            )

                    # ---- V projection: V[kv, dv] = sum_h xv[h, kv] * wvT[h, dv] ----
                    # (kv on partitions so V can be the attn@V stationary operand)
                    for grp in range(half * 2, half * 2 + 2):  # 4 groups of 8 kv chunks
                        psv = [pp.tile([128, 512], F32, tag="psk", name=f"psv_{c}")[:, :DS] for c in range(8)]
                        for j in range(4):
                            xvt = xs.tile([128, 2, 1024], BF16, tag="xv")
                            nc.gpsimd.dma_start(xvt[:], xv[:, grp, j, :, :])
                            for i in range(2):
                                h = j * 2 + i
                                for c in range(8):
                                    nc.tensor.matmul(
                                        psv[c][:],
                                        xvt[:, i, c * 128 : (c + 1) * 128],
                                        wv_sb[:, h, :],
                                        start=(h == 0),
                                        stop=(h == HC - 1),
                                    )
                        for c in range(8):
                            nc.vector.tensor_copy(
                                Vsb[:, grp * 8 + c, :, 0:HEAD_DIM],
                                psv[c].rearrange("p (hh d) -> p hh d", hh=HPC),
                            )

                # ---- Q projection ----
                psq = [
                    [pp.tile([128, 512], F32, tag="psk", name=f"psq_{dq}_{t}") for t in range(4)]
                    for dq in range(DC)
                ]
                for j in range(4):
                    xqt = xs.tile([128, 2, 2048], BF16, tag="xk")
                    eng = nc.sync if j % 2 == 0 else nc.scalar
                    eng.dma_start(xqt[:], xq[:, j, :, :])
                    for i in range(2):
                        h = j * 2 + i
                        for dq in range(DC):
                            for t in range(4):
                                nc.tensor.matmul(
                                    psq[dq][t][:],
                                    wq_sb[:, h, dq * 128 : (dq + 1) * 128],
                                    xqt[:, i, t * 512 : (t + 1) * 512],
                                    start=(h == 0),
                                    stop=(h == HC - 1),
                                )
                for h in range(HPC):
                    pb = (h % 2) * 64
                    for t in range(4):
                        nc.vector.tensor_copy(
                            QT[pb : pb + 64, h, t * 512 : (t + 1) * 512],
                            psq[h // 2][t][pb : pb + 64, :],
                        )

            # ---- Attention + deferred normalization/output projection ----
            with (
                tc.tile_pool(name="attn_sb", bufs=8) as asb,
                tc.tile_pool(name="exp_scr", bufs=1) as sxp,
                tc.tile_pool(name="den_sb", bufs=2) as dsb,
                tc.tile_pool(name="xmv_sb", bufs=2) as xsb,
                tc.tile_pool(name="out_sb", bufs=4) as osb,
                tc.tile_pool(name="pstg", bufs=2, space="PSUM") as pstg,
                tc.tile_pool(name="pstgd", bufs=1, space="PSUM") as pstgd,
                tc.tile_pool(name="px", bufs=1, space="PSUM") as px,
            ):

                def recip(qh):
                    nc.vector.reciprocal_approx_accurate(
                        out=recA[qh][:], in_=denC[qh][:], scratch=recS[:]
                    )
                    nc.vector.tensor_copy(recR[qh][:], recA[qh][:])

                def normalize(qh):
                    """Broadcast 1/den and normalize Xraw -> Xn for q-half
                    qh. Cheap (4 small f32r matmuls + 2 multiplies); when
                    embedded a few chunks into the next q-half its chain
                    hides under the attention stream. The bc tiles borrow
                    the stg rotation (PSUM is fully booked)."""
                    for hc in range(2):
                        bc = pstg.tile([128, 1024], F32, tag="stg", name="bc")
                        for t in range(2):
                            nc.tensor.matmul(
                                bc[:, t * 512 : (t + 1) * 512],
                                selC[:, hc, :],
                                recR[qh][:, t * 512 : (t + 1) * 512],
                                start=True,
                                stop=True,
                            )
                        nc.vector.tensor_tensor(
                            Xn[:, hc, qh * 1024 : (qh + 1) * 1024],
                            Xraw[:, hc, qh * 1024 : (qh + 1) * 1024],
                            bc[:],
                            mybir.AluOpType.mult,
                        )

                def outproj(t):
                    for oc in range(HIDDEN // 128):
                        pw = pstg.tile([128, 1024], F32, tag="stg", name="pso")
                        p = pw[:, 0:512]
                        for dv in range(DC):
                            nc.tensor.matmul(
                                p[:],
                                wo_sb[:, dv, oc * 128 : (oc + 1) * 128],
                                Xn[:, dv, t * 512 : (t + 1) * 512],
                                start=(dv == 0),
                                stop=(dv == DC - 1),
                            )
                        ot = osb.tile([128, 512], BF16, tag="outsb")
                        if oc % 2 == 0:
                            nc.vector.tensor_copy(ot[:], p[:])
                        else:
                            nc.scalar.copy(ot[:], p[:])
                        eng = (nc.sync, nc.scalar, nc.gpsimd)[oc % 3]
                        eng.dma_start(
                            outT[oc * 128 : (oc + 1) * 128, t * 512 : (t + 1) * 512],
                            ot[:],
                        )

                def fast_exp(eng, tag, stg, eS):
                    """exp on the DVE via corrected Schraudolph (see top):
                    5 elementwise instructions."""
                    yi = sxp.tile([128, 1024], I32, tag=f"{tag}_yi")
                    zf = sxp.tile([128, 1024], F32, tag=f"{tag}_zf")
                    pf = sxp.tile([128, 1024], F32, tag=f"{tag}_pf")
                    wf = sxp.tile([128, 1024], F32, tag=f"{tag}_wf")
                    eng.tensor_scalar(
                        yi[:], stg[:], EXP_A * SCALE, EXP_B,
                        mybir.AluOpType.mult, mybir.AluOpType.add,
                    )
                    eng.tensor_scalar(
                        zf.bitcast(I32)[:], yi[:], mskM[:], mskE[:],
                        mybir.AluOpType.bitwise_and, mybir.AluOpType.bitwise_or,
                    )
                    eng.tensor_scalar(
                        pf[:], zf[:], EXP_C1, EXP_C2,
                        mybir.AluOpType.mult, mybir.AluOpType.add,
                    )
                    eng.tensor_tensor(wf[:], pf[:], zf[:], mybir.AluOpType.mult)
                    eng.scalar_tensor_tensor(
                        eS[:], wf[:], 1.0, yi.bitcast(F32)[:],
                        mybir.AluOpType.add, mybir.AluOpType.mult,
                    )

                for qh in range(2):
                    q0 = qh * 1024
                    for h in range(HPC):
                        hc = h // 2
                        pb = (h % 2) * 64  # partition base of this head's dims
                        X = px.tile([128, 2, 512], F32, tag="x")
                        den = dsb.tile([66, 2, 512], F32, tag="den")
                        pend = []  # deferred attn@V: (chunk, eS, ready-at)
                        emitted = 0

                        def attnv(c, eS, last):
                            nonlocal emitted
                            for t in range(2):
                                nc.tensor.matmul(
                                    X[0:65, t, :],
                                    Vsb[:, c, h, :],
                                    eS[:, t * 512 : (t + 1) * 512],
                                    start=(emitted == 0),
                                    stop=last,
                                )
                            emitted += 1

                        for c in range(KVC):
                            # a few chunks into (qh=1, h=0), squeeze the qh=0
                            # normalization into the stream: its dependency
                            # chain (denC DMA -> recip -> bcast -> mult)
                            # resolves under the attention pipeline.
                            if qh == 1 and h == 0 and c == 4:
                                recip(0)
                            if qh == 1 and h == 0 and c == 8:
                                normalize(0)
                            # 5 of 32 chunks' exp runs on the DVE instead of
                            # the scalar engine (GPSIMD elementwise is ~8x
                            # slower than its cost model - useless here);
                            # those chunks get their own PSUM tile so the
                            # ACT-chunk stg rotation never waits on the slow
                            # chain, and their attn@V is deferred.
                            slow = c in (3, 9, 15, 21, 27)
                            if slow:
                                stg = pstgd.tile([128, 1024], F32, tag="stgd")
                            else:
                                stg = pstg.tile([128, 1024], F32, tag="stg")
                            for t in range(2):
                                nc.tensor.matmul(
                                    stg[:, t * 512 : (t + 1) * 512],
                                    KT[:, hc, c * 128 : (c + 1) * 128],
                                    QT[:, h, q0 + t * 512 : q0 + (t + 1) * 512],
                                    start=True,
                                    stop=True,
                                )
                            eS = asb.tile([128, 1024], BF16, tag="expS")
                            if slow:
                                fast_exp(nc.vector, "d", stg, eS)
                                pend.append((c, eS, c + 5))
                            else:
                                nc.scalar.activation(
                                    out=eS[:],
                                    in_=stg[:],
                                    func=mybir.ActivationFunctionType.Exp,
                                    scale=SCALE,
                                )
                                pend.append((c, eS, c + 1))
                            for item in [p for p in pend if p[2] <= c]:
                                pend.remove(item)
                                attnv(item[0], item[1], False)
                        while pend:
                            pc, pe, _ = pend.pop(0)
                            attnv(pc, pe, not pend)
                        for t in range(2):
                            q1 = q0 + t * 512
                            if pb == 0:
                                nc.vector.tensor_copy(
                                    Xraw[0:64, hc, q1 : q1 + 512], X[0:64, t, :]
                                )
                            else:
                                sc = xsb.tile([64, 512], BF16, tag="xmv")
                                nc.vector.tensor_copy(sc[:], X[0:64, t, :])
                                nc.sync.dma_start(
                                    Xraw[64:128, hc, q1 : q1 + 512], sc[:]
                                )
                        nc.vector.tensor_copy(den[64:65, :, :], X[64:65, :, :])
                        nc.sync.dma_start(denC[qh][h : h + 1, :], den[64:65, :, :])

                # End block: the q-half-0 output columns are long ready and
                # keep the PE busy while the q-half-1 normalize chain
                # (recip on DVE, then bcast/mult) resolves in parallel.
                recip(1)
                outproj(0)
                outproj(1)
                normalize(1)
                outproj(2)
                outproj(3)

    nc.finalize()
    return nc


_PROGRAM = None


def _program():
    global _PROGRAM
    if _PROGRAM is None:
        _PROGRAM = _build_program()
    return _PROGRAM


def _pack_xq(xT):  # [1024, 2048] -> [128, 4, 2, 2048]
    return np.ascontiguousarray(xT.reshape(4, 2, 128, 2048).transpose(2, 0, 1, 3))


def _pack_xk(xT):  # [1024, 4096] -> [128, 2, 4, 2, 2048]
    return np.ascontiguousarray(
        xT.reshape(4, 2, 128, 2, 2048).transpose(2, 3, 0, 1, 4)
    )


def _pack_xv(xT):  # [1024, 4096] -> [128, 4, 4, 2, 1024]
    return np.ascontiguousarray(
        xT.reshape(4, 2, 128, 4, 1024).transpose(2, 3, 0, 1, 4)
    )


def _make_sel():
    sel = np.zeros((4, 2, 128), np.float32)
    for hc in range(2):
        sel[2 * hc, hc, 0:64] = 1.0
        sel[2 * hc + 1, hc, 64:128] = 1.0
    return sel


def _pack_w(wT):  # [1024, M] -> [128, chunks, M]: (c p) m -> p c m
    m = wT.shape[1]
    return np.ascontiguousarray(wT.reshape(-1, 128, m).transpose(1, 0, 2))


def _shard_inputs(query_states, key_states, value_states, Wq, Wk, Wv, Wo):
    # per-batch packed bf16 activations, shared by the 4 cores of a batch
    xqs = [_pack_xq(query_states[b].T.astype(NP_BF16)) for b in range(B)]
    xks = [_pack_xk(key_states[b].T.astype(NP_BF16)) for b in range(B)]
    xvs = [_pack_xv(value_states[b].T.astype(NP_BF16)) for b in range(B)]
    sel = _make_sel()
    in_maps = []
    for core in range(N_CORES):
        b = core // HPC
        hg = core % HPC
        s = slice(hg * DS, (hg + 1) * DS)
        in_maps.append(
            {
                "xq": xqs[b],
                "xk": xks[b],
                "xv": xvs[b],
                "wqT": _pack_w(Wq[s, :].T.astype(NP_BF16)),
                "wkT": _pack_w(Wk[s, :].T.astype(NP_BF16)),
                "wvT": _pack_w(Wv[s, :].T.astype(NP_BF16)),
                "woT": _pack_w(Wo[:, s].T.astype(NP_BF16)),
                "selD": sel,
            }
        )
    return in_maps


def _gather_output(results):
    out = np.empty((B, QL, HIDDEN), np.float32)
    for b in range(B):
        acc = results[b * HPC]["outT"].astype(np.float32)
        for i in range(1, HPC):
            acc = acc + results[b * HPC + i]["outT"].astype(np.float32)
        out[b] = acc.T
    return out


def run_sharded(inputs, trace=False, tmpdir=None):
    """Run the SPMD kernel; returns (full_output, BassKernelResults)."""
    arrs = {k: np.asarray(v, dtype=np.float32) for k, v in inputs.items()}
    in_maps = _shard_inputs(
        arrs["query_states"],
        arrs["key_states"],
        arrs["value_states"],
        arrs["Wq"],
        arrs["Wk"],
        arrs["Wv"],
        arrs["Wo"],
    )
    res = run_bass_kernel_spmd(
        _program(), in_maps, list(range(N_CORES)), trace=trace, tmpdir=tmpdir
    )
    return _gather_output(res.results), res


def kernel(**inputs):
    out, _ = run_sharded(inputs)
    return out
